# revision 23
# baseline (speedup 1.0000x reference)
"""TRN2 Bass kernel for gnn_message_passing (nn_Model_34823594836411), v2.

Math (matches reference.py):
  per edge e: rel = pos[dst] - pos[src]; sh1 = rel / max(|rel|, 1e-12)
  out[n, 0]   = w0 * f[n] * min(c_n, 1)
  out[n, 1:4] = w1 * f[n] * segsum(sh1)_n / max(c_n, 1)
where f = node_feat[:, 0] and c_n = in-degree of node n.

Design (the end-to-end wall is dominated by the ~50-85 MB/s axon host->
device link, so everything minimizes shipped bytes; on-device exec is
~1.5ms):
  * dst-shard nodes across 8 cores (12544/core). Each node owns a padded
    row of C slots; the only random access is the src-position gather via
    the ANT dma_gather SWDGE ucode over 256B-stride records of 4 nodes.
  * The 6.4MB/core gather table is NOT shipped: positions are shipped
    densely, 1/8-sharded per core (150KB), AllGathered on device, and the
    record table is built on device (vector 12->64 expand + one DMA).
  * Nodes are degree-sorted per core into capacity classes
    (C in {72,48,40,36,32,28} instead of uniform 64), cutting padded
    slots/node from 64 to ~35.7. Host un-permutes the output.
  * Slot C-1 of every row is a self-edge, so the on-chip select also
    yields the node's own position (no pdst input); self slots contribute
    exactly zero (rel=0 -> sh=0).
  * 2-bit record-select codes ship packed 4-per-byte; in-degrees ship as
    uint16; both are unpacked/converted on device.
  * The jit executable is AOT-compiled once and cached; output donation
    buffers are created on device (never shipped) and recycled across
    calls; output shards are fetched with parallel RPCs.
"""
import time
from contextlib import ExitStack

import numpy as np

import concourse.bacc as bacc
import concourse.bass as bass
import concourse.mybir as mybir
from concourse import library_config
from concourse._compat import exact_div

N_NODES = 100000
N_EDGES = 3200000
NC = 8
P = 128
NPC = 12544            # nodes per core (98 blocks of 128); 8*12544 = 100352
B = NPC // P           # 98 blocks
NT = NC * NPC          # padded node table size
NREC = NT // 4         # 4-node records in the position table
EPS2 = 1e-24
CALL_IDX = 1024        # gather idxs per dma_gather call (ring-capacity safe)
MAX_CH_COLS = 896      # SBUF budget for per-chunk tiles
# (capacity C, num 128-node blocks) from highest degree to lowest; sum of
# blocks must be B. Slot C-1 of every node is reserved as a self-edge (so
# the on-chip select also yields the node's own position -> no pdst input);
# usable capacity is C-1. Sized for Poisson(32) degrees with >=5 sigma
# margin at every boundary.
CLASSES = ((72, 1), (48, 10), (40, 16), (36, 26), (32, 27), (28, 18))
POS_SHARDED = True     # ship positions 1/8-sharded + AllGather on device


def set_mini(n_nodes, nc_, npc, classes, call_idx=128, max_ch_cols=64):
    """Shrink the problem for CoreSim debugging."""
    global N_NODES, NC, NPC, B, NT, NREC, CLASSES, CALL_IDX, MAX_CH_COLS
    N_NODES, NC, NPC = n_nodes, nc_, npc
    B = NPC // P
    NT = NC * NPC
    NREC = NT // 4
    CLASSES = classes
    CALL_IDX = call_idx
    MAX_CH_COLS = max_ch_cols


F32 = mybir.dt.float32
I16 = mybir.dt.int16
U8 = mybir.dt.uint8


def _ap(t, off, dims):
    return bass.AP(t, off, dims)


def _chunk_plan(classes):
    """Static chunk descriptors: (C, nb, bg0, col0, cols)."""
    chunks = []
    bg = 0
    col = 0
    for C, nblocks in classes:
        left = nblocks
        while left > 0:
            nb = min(left, MAX_CH_COLS // C)
            if C % 8 and nb % 2 and nb > 1:
                nb -= 1          # keep nb*C a multiple of 8 (CALL_IDX slicing)
            assert nb >= 1 and (nb * C) % 8 == 0, (C, nb)
            chunks.append((C, nb, bg, col, nb * C))
            bg += nb
            col += nb * C
            left -= nb
    assert bg == sum(nb for _, nb in classes)
    return chunks, col


def dma_gather_raw(gpsimd, out_ap, in_ap, idxs_ap, num_idxs, elem_size,
                   elem_step, queue_num=0):
    """Non-transpose DRAM-source InstDMAGatherAnt without the 256B-elem
    assert: out[i % 128, i // 128, :] = table[idx[i], :elem_size]."""
    stride_bytes_256 = exact_div(elem_step * 4, 256)
    return gpsimd.add_instruction(
        mybir.InstDMAGatherAnt(
            name=gpsimd.bass.get_next_instruction_name(),
            ins=[
                *gpsimd.lower_ap_dma(in_ap, for_custom_bir_dma=True),
                gpsimd.lower_ap(idxs_ap),
                gpsimd.lower_val_access(gpsimd.to_reg(num_idxs)),
            ],
            outs=[gpsimd.lower_ap(out_ap)],
            transpose=False,
            num_idxs=num_idxs,
            elem_size=elem_size,
            stride_bytes_256=stride_bytes_256,
            gen_mode=0,
            single_packet=True,
            queue_num=queue_num,
            sbuf_tokens_per_rank=0,
            sbuf_free_dim_per_rank=0,
            sbuf_free_dim_pad_per_rank=0,
            sbuf_byte_offset=0,
        )
    )


_PROG_CACHE = {}
LAST_DEVICE_WALL_S = None


def build_program(classes, pos_sharded):
    key = (classes, pos_sharded, NPC, CALL_IDX, MAX_CH_COLS)
    if key in _PROG_CACHE:
        return _PROG_CACHE[key]

    AL = mybir.AluOpType
    chunks, cols = _chunk_plan(classes)
    max_cols = max(c[4] for c in chunks)
    pf_cols = exact_div(NT * 3, P)   # dense position table as a [128, x] tile
    qrec = exact_div(NREC, P)        # records per partition in the build image

    nc = bacc.Bacc("TRN2", num_swdge_queues=4, num_devices=NC)
    _eps_t = nc.alloc_sbuf_tensor("const-float32-eps2", [128, 1], F32)
    nc.gpsimd.memset(_eps_t.ap(), EPS2)
    nc.const_aps.aps[(F32, EPS2)] = _eps_t.ap()
    nc.all_engine_barrier()

    # all inputs ship as TWO per-core arrays (one int16, one f32) to
    # minimize per-array host->device placement overhead over axon.
    # int16 layout: [16*W idx stream | 128*(cols//8) packed codes | 128*B
    # degrees]; f32 layout: [positions | 128*B node_feat | 128*4 weights]
    W = cols * P // 16
    OFF_B = 16 * W
    OFF_C = OFF_B + 128 * (cols // 8)
    M16 = OFF_C + 128 * B
    plen = (NPC if pos_sharded else NT) * 3
    OFF_N = plen
    OFF_W = OFF_N + 128 * B
    MF = OFF_W + 128 * 4
    meg16 = nc.dram_tensor("meg16", [M16], I16, kind="ExternalInput")
    megf = nc.dram_tensor("megf", [MF], F32, kind="ExternalInput")
    if pos_sharded:
        pbounce = nc.dram_tensor("pbounce", [NPC, 3], F32)
        pfull = nc.dram_tensor("pfull", [NT, 3], F32)
    else:
        pfull = None                    # dense positions read from megf
    ptab = nc.dram_tensor("ptab", [NREC, 64], F32)
    # output ships fp16 scaled by 2^12 (exact) to halve the fetch bytes;
    # host unscales. fp16 rounding is a bounded ~5e-4 RELATIVE error (no
    # cancellation amplification), far inside the 2e-2 gate; the scale
    # keeps all representable values out of subnormal-flush range.
    # every core's fp16 result is AllGathered on device (NeuronLink is
    # ~free) so the host fetches ONE device's shard with a single RPC
    # instead of 8 latency-bound ones
    F16 = mybir.dt.float16
    OUT_ELEMS = 128 * B * 4
    obounce = nc.dram_tensor("obounce", [128, B, 4], F16)
    ogath = nc.dram_tensor("ogath", [NC, OUT_ELEMS], F16)
    out = nc.dram_tensor("out", [NC, OUT_ELEMS], F16, kind="ExternalOutput")

    tab_ap = _ap(ptab, 0, [[64, NREC], [1, 12]])

    # deterministic semaphore schedule, computed identically for every engine
    n_chunks = len(chunks)
    calls_of = [exact_div(c[4] * P, CALL_IDX) for c in chunks]
    g_setup = 16 * (6 if pos_sharded else 5)   # setup DMAs (see gpsimd below)
    g_after = []
    q_after = []
    g = g_setup
    qc = [0, 0, 0, 0]
    for ch in range(n_chunks):
        g += 16 * 9                      # 8 idx-group DMAs + cpk DMA
        g_after.append(g)
        for k in range(calls_of[ch]):
            qc[k % 4] += 16
        q_after.append(tuple(qc))

    with ExitStack() as _st:
        idx_sb = _st.enter_context(
            nc.sbuf_tensor("idx_sb", [128, max_cols * P // 16], I16))
        sbE = _st.enter_context(nc.sbuf_tensor("sbE", [128, qrec, 64], F32))
        cpk_sb = _st.enter_context(nc.sbuf_tensor("cpk_sb", [128, max_cols // 8], I16))
        cdt_sb = _st.enter_context(nc.sbuf_tensor("cdt_sb", [128, max_cols // 8], I16))
        cd_sb = _st.enter_context(nc.sbuf_tensor("cd_sb", [128, max_cols], I16))
        rec_sb = _st.enter_context(nc.sbuf_tensor("rec_sb", [128, max_cols, 12], F32))
        mk_sb = _st.enter_context(nc.sbuf_tensor("mk_sb", [128, 4, max_cols], F32))
        pa_sb = _st.enter_context(nc.sbuf_tensor("pa_sb", [128, max_cols, 3], F32))
        pb_sb = _st.enter_context(nc.sbuf_tensor("pb_sb", [128, max_cols, 3], F32))
        ss_sb = _st.enter_context(nc.sbuf_tensor("ss_sb", [128, max_cols], F32))
        inv_sb = _st.enter_context(nc.sbuf_tensor("inv_sb", [128, max_cols], F32))
        pdst_sb = _st.enter_context(nc.sbuf_tensor("pdst_sb", [128, B, 3], F32))
        sums_sb = _st.enter_context(nc.sbuf_tensor("sums_sb", [128, B, 3], F32))
        cnt_sb = _st.enter_context(nc.sbuf_tensor("cnt_sb", [128, B], I16))
        cntf_sb = _st.enter_context(nc.sbuf_tensor("cntf_sb", [128, B], F32))
        nf_sb = _st.enter_context(nc.sbuf_tensor("nf_sb", [128, B], F32))
        w_sb = _st.enter_context(nc.sbuf_tensor("w_sb", [128, 4], F32))
        o_sb = _st.enter_context(nc.sbuf_tensor("o_sb", [128, B, 4], F32))
        o16_sb = _st.enter_context(
            nc.sbuf_tensor("o16_sb", [128, B, 4], mybir.dt.float16))
        t0_sb = _st.enter_context(nc.sbuf_tensor("t0_sb", [128, B], F32))
        t1_sb = _st.enter_context(nc.sbuf_tensor("t1_sb", [128, B], F32))
        g_sem = _st.enter_context(nc.semaphore("g_sem"))
        q0_sem = _st.enter_context(nc.semaphore("q0_sem"))
        q1_sem = _st.enter_context(nc.semaphore("q1_sem"))
        q2_sem = _st.enter_context(nc.semaphore("q2_sem"))
        q3_sem = _st.enter_context(nc.semaphore("q3_sem"))
        v_sem = _st.enter_context(nc.semaphore("v_sem"))
        a_sem = _st.enter_context(nc.semaphore("a_sem"))
        c_sem = _st.enter_context(nc.semaphore("c_sem"))
        b_sem = _st.enter_context(nc.semaphore("b_sem"))
        block = _st.enter_context(nc.Block())

        # dense pfull viewed as a [128, pf_cols] SBUF tile in linear order:
        # partition p holds f32 elements [p*pf_cols, (p+1)*pf_cols)
        pf_flat_sb = _ap(pa_sb, 0, [[max_cols * 3, 128], [1, pf_cols]])
        pf_flat_dram = _ap(pfull if pos_sharded else megf, 0,
                           [[pf_cols, 128], [1, pf_cols]])
        # table build: vector expands 12->64 f32/record into sbE (partition p
        # holds records [p*qrec, (p+1)*qrec)), then one fully-contiguous DMA
        # writes ptab (row r payload lands at 256B*r; cols 12..63 are
        # garbage the gather never reads)
        exp_src = _ap(pa_sb, 0, [[max_cols * 3, 128], [12, qrec], [1, 12]])
        exp_dst = _ap(sbE, 0, [[qrec * 64, 128], [64, qrec], [1, 12]])
        tab_dst = _ap(ptab, 0, [[qrec * 64, 128], [1, qrec * 64]])
        tab_src = _ap(sbE, 0, [[qrec * 64, 128], [1, qrec * 64]])

        @block.gpsimd
        def _(gpsimd):
            gpsimd.load_library(library_config.mlp)
            ng = 0
            if pos_sharded:
                gpsimd.dma_start(
                    pbounce[:], _ap(megf, 0, [[3, NPC], [1, 3]])
                ).then_inc(g_sem, 16)
                ng += 16
                gpsimd.wait_ge(g_sem, ng)
                gpsimd.collective_compute(
                    "AllGather",
                    AL.bypass,
                    replica_groups=[list(range(NC))],
                    ins=[pbounce.ap().opt()],
                    outs=[pfull.ap().opt()],
                ).then_inc(c_sem, 1)
                gpsimd.wait_ge(c_sem, 1)
            gpsimd.dma_start(pf_flat_sb, pf_flat_dram).then_inc(g_sem, 16)
            ng += 16
            gpsimd.wait_ge(b_sem, 1)        # vector expanded sbE
            gpsimd.dma_start(tab_dst, tab_src).then_inc(g_sem, 16)
            ng += 16
            gpsimd.dma_start(
                cnt_sb[:], _ap(meg16, OFF_C, [[B, 128], [1, B]])
            ).then_inc(g_sem, 16)
            gpsimd.dma_start(
                nf_sb[:], _ap(megf, OFF_N, [[B, 128], [1, B]])
            ).then_inc(g_sem, 16)
            gpsimd.dma_start(
                w_sb[:], _ap(megf, OFF_W, [[4, 128], [1, 4]])
            ).then_inc(g_sem, 16)
            ng += 16 * 3
            assert ng == g_setup
            # all setup DMAs (incl. the table build write) must land before
            # the first gather reads ptab / vector overwrites pa_sb
            gpsimd.wait_ge(g_sem, g_setup)
            q_sems = (q0_sem, q1_sem, q2_sem, q3_sem)
            for ch, (C, nb, bg0, col0, ccols) in enumerate(chunks):
                if ch >= 1:
                    # chunk buffers are single-buffered: wait for compute
                    gpsimd.wait_ge(v_sem, ch)
                iw = ccols * P // 16
                ioff = col0 * P // 16
                for grp in range(8):
                    # replicate the wrapped idx stream into each
                    # 16-partition group on device
                    gpsimd.dma_start(
                        idx_sb[16 * grp:16 * (grp + 1), :iw],
                        _ap(meg16, ioff, [[W, 16], [1, iw]]),
                    ).then_inc(g_sem, 16)
                gpsimd.dma_start(
                    cpk_sb[:, :ccols // 8],
                    _ap(meg16, OFF_B + col0 // 8,
                        [[cols // 8, 128], [1, ccols // 8]]),
                ).then_inc(g_sem, 16)
                gpsimd.wait_ge(g_sem, g_after[ch])
                ncalls = calls_of[ch]
                ccall = CALL_IDX // P    # record columns written per call
                for k in range(ncalls):
                    dma_gather_raw(
                        gpsimd,
                        rec_sb[:, k * ccall:(k + 1) * ccall, :],
                        tab_ap,
                        idx_sb[:, k * (CALL_IDX // 16):(k + 1) * (CALL_IDX // 16)],
                        num_idxs=CALL_IDX, elem_size=12, elem_step=64,
                        queue_num=k % 4,
                    ).then_inc(q_sems[k % 4], 16)
            gpsimd.wait_ge(v_sem, n_chunks + 1)
            gpsimd.dma_start(obounce[:], o16_sb[:]).then_inc(g_sem, 16)
            gpsimd.wait_ge(g_sem, g_after[-1] + 16)
            gpsimd.collective_compute(
                "AllGather",
                AL.bypass,
                replica_groups=[list(range(NC))],
                ins=[obounce.ap().opt()],
                outs=[ogath.ap().opt()],
            ).then_inc(c_sem, 1)
            gpsimd.wait_ge(c_sem, 2 if pos_sharded else 1)
            gpsimd.dma_start(out[:], ogath[:]).then_inc(g_sem, 16)
            gpsimd.wait_ge(g_sem, g_after[-1] + 32)
            for qi, q in enumerate(q_sems):
                gpsimd.wait_ge(q, q_after[-1][qi])

        @block.vector
        def _(vector):
            # expand the dense positions into the 256B-stride record image
            vector.memset(sbE[:], 0.0)
            vector.drain()
            vector.wait_ge(g_sem, 32 if pos_sharded else 16)
            vector.tensor_copy(out=exp_dst, in_=exp_src)
            vector.drain().then_inc(b_sem, 1)
            for ch, (C, nb, bg0, col0, ccols) in enumerate(chunks):
                vector.wait_ge(g_sem, g_after[ch])
                for qi, q in enumerate((q0_sem, q1_sem, q2_sem, q3_sem)):
                    if q_after[ch][qi]:
                        vector.wait_ge(q, q_after[ch][qi])
                qcols = ccols // 8
                # unpack the 2-bit codes: cd[:, 8q+r] = (cpk[:, q] >> 2r) & 3
                for r in range(8):
                    srcap = cpk_sb[:, :qcols]
                    if r > 0:
                        vector.tensor_scalar(
                            out=cdt_sb[:, :qcols], in0=cpk_sb[:, :qcols],
                            scalar1=2 * r, scalar2=None,
                            op0=AL.logical_shift_right)
                        vector.drain()
                        srcap = cdt_sb[:, :qcols]
                    vector.tensor_scalar(
                        out=_ap(cd_sb, r, [[max_cols, 128], [8, qcols]]),
                        in0=srcap, scalar1=3, scalar2=None,
                        op0=AL.bitwise_and)
                    vector.drain()
                # derive the four 0/1 masks from the code plane
                for kk in range(4):
                    vector.tensor_scalar(
                        out=_ap(mk_sb, kk * max_cols,
                                [[4 * max_cols, 128], [1, ccols]]),
                        in0=cd_sb[:, :ccols], scalar1=kk, scalar2=None,
                        op0=AL.is_equal)
                vector.drain()
                # exact select: psrc = sum_k rec_k * mask_k (three terms are
                # exact zeros, so the sum is bit-exact)
                def mk(kk):
                    return _ap(mk_sb, kk * max_cols,
                               [[4 * max_cols, 128], [1, ccols], [0, 3]])
                vector.tensor_tensor(out=pa_sb[:, :ccols, :],
                                     in0=rec_sb[:, :ccols, 0:3],
                                     in1=mk(0), op=AL.mult)
                for kk in range(1, 4):
                    vector.tensor_tensor(out=pb_sb[:, :ccols, :],
                                         in0=rec_sb[:, :ccols, 3 * kk:3 * kk + 3],
                                         in1=mk(kk), op=AL.mult)
                    vector.drain()
                    vector.tensor_tensor(out=pa_sb[:, :ccols, :],
                                         in0=pa_sb[:, :ccols, :],
                                         in1=pb_sb[:, :ccols, :],
                                         op=AL.add)
                    vector.drain()
                # slot C-1 is a self-edge: psrc there is this node's own
                # position; stash it as pdst for the chunk's blocks
                vector.tensor_copy(
                    out=_ap(pdst_sb, bg0 * 3, [[B * 3, 128], [3, nb], [1, 3]]),
                    in_=_ap(pa_sb, (C - 1) * 3,
                            [[max_cols * 3, 128], [C * 3, nb], [1, 3]]))
                vector.drain()
                # rel = pdst - psrc (in place, 4D APs)
                pd = _ap(pdst_sb, bg0 * 3,
                         [[B * 3, 128], [3, nb], [0, C], [1, 3]])
                pa4 = _ap(pa_sb, 0,
                          [[max_cols * 3, 128], [C * 3, nb], [3, C], [1, 3]])
                vector.tensor_tensor(out=pa4, in0=pd, in1=pa4, op=AL.subtract)
                vector.drain()
                # ss = sum of squares over components
                vector.tensor_tensor(out=pb_sb[:, :ccols, :],
                                     in0=pa_sb[:, :ccols, :],
                                     in1=pa_sb[:, :ccols, :], op=AL.mult)
                vector.drain()
                sq_x = _ap(pb_sb, 0, [[max_cols * 3, 128], [3, ccols]])
                sq_y = _ap(pb_sb, 1, [[max_cols * 3, 128], [3, ccols]])
                sq_z = _ap(pb_sb, 2, [[max_cols * 3, 128], [3, ccols]])
                vector.tensor_tensor(out=ss_sb[:, :ccols], in0=sq_x, in1=sq_y,
                                     op=AL.add)
                vector.drain()
                vector.tensor_tensor(out=ss_sb[:, :ccols], in0=ss_sb[:, :ccols],
                                     in1=sq_z, op=AL.add)
                vector.drain().then_inc(a_sem, 1)
                # sh = rel * rsqrt(ss + eps^2) once ACT publishes inv
                vector.wait_ge(a_sem, 2 * ch + 2)
                vector.reciprocal(out=inv_sb[:, :ccols], in_=inv_sb[:, :ccols])
                vector.drain()
                invb = _ap(inv_sb, 0, [[max_cols, 128], [1, ccols], [0, 3]])
                vector.tensor_tensor(out=pa_sb[:, :ccols, :],
                                     in0=pa_sb[:, :ccols, :], in1=invb,
                                     op=AL.mult)
                vector.drain()
                # reduce over C: halve while even, else fold last col into 0
                width = C
                while width > 1:
                    if width % 2 == 0:
                        half = width // 2
                        a_lo = _ap(pa_sb, 0,
                                   [[max_cols * 3, 128], [C * 3, nb],
                                    [3, half], [1, 3]])
                        a_hi = _ap(pa_sb, half * 3,
                                   [[max_cols * 3, 128], [C * 3, nb],
                                    [3, half], [1, 3]])
                        vector.tensor_tensor(out=a_lo, in0=a_lo, in1=a_hi,
                                             op=AL.add)
                        vector.drain()
                        width = half
                    else:
                        a_l0 = _ap(pa_sb, 0,
                                   [[max_cols * 3, 128], [C * 3, nb], [1, 3]])
                        a_lst = _ap(pa_sb, (width - 1) * 3,
                                    [[max_cols * 3, 128], [C * 3, nb], [1, 3]])
                        vector.tensor_tensor(out=a_l0, in0=a_l0, in1=a_lst,
                                             op=AL.add)
                        vector.drain()
                        width -= 1
                dst_sums = _ap(sums_sb, bg0 * 3,
                               [[B * 3, 128], [3, nb], [1, 3]])
                src_sums = _ap(pa_sb, 0,
                               [[max_cols * 3, 128], [C * 3, nb], [1, 3]])
                vector.tensor_copy(out=dst_sums, in_=src_sums)
                vector.drain().then_inc(v_sem, 1)
            # final combine
            vector.tensor_copy(out=cntf_sb[:], in_=cnt_sb[:])
            vector.drain()
            vector.tensor_scalar_min(out=t0_sb[:], in0=cntf_sb[:], scalar1=1.0)
            vector.tensor_scalar_max(out=t1_sb[:], in0=cntf_sb[:], scalar1=1.0)
            vector.drain()
            vector.reciprocal(out=t1_sb[:], in_=t1_sb[:])
            vector.drain()
            vector.tensor_tensor(out=t1_sb[:], in0=t1_sb[:], in1=nf_sb[:],
                                 op=AL.mult)
            vector.drain()
            o0 = _ap(o_sb, 0, [[B * 4, 128], [4, B]])
            w0b = _ap(w_sb, 0, [[4, 128], [0, B]])
            vector.tensor_tensor(out=o0, in0=t0_sb[:], in1=nf_sb[:], op=AL.mult)
            vector.drain()
            vector.tensor_tensor(out=o0, in0=o0, in1=w0b, op=AL.mult)
            vector.drain()
            for c in range(3):
                oc = _ap(o_sb, 1 + c, [[B * 4, 128], [4, B]])
                sc = _ap(sums_sb, c, [[B * 3, 128], [3, B]])
                wcb = _ap(w_sb, 1 + c, [[4, 128], [0, B]])
                vector.tensor_tensor(out=oc, in0=sc, in1=t1_sb[:], op=AL.mult)
                vector.drain()
                vector.tensor_tensor(out=oc, in0=oc, in1=wcb, op=AL.mult)
                vector.drain()
            # scaled fp16 cast for the half-size output fetch
            vector.tensor_scalar(out=o16_sb[:], in0=o_sb[:],
                                 scalar1=4096.0, scalar2=None, op0=AL.mult)
            vector.drain().then_inc(v_sem, 1)

        @block.scalar
        def _(scalar):
            for ch, (C, nb, bg0, col0, ccols) in enumerate(chunks):
                scalar.wait_ge(a_sem, 2 * ch + 1)
                scalar.activation(
                    out=inv_sb[:, :ccols], in_=ss_sb[:, :ccols],
                    func=mybir.ActivationFunctionType.Sqrt,
                    bias=EPS2, scale=1.0,
                ).then_inc(a_sem, 1)

    nc.compile()
    _PROG_CACHE[key] = nc
    return nc


def _class_layout(classes):
    bounds = []
    a = 0
    for C, nblocks in classes:
        m = nblocks * P
        bounds.append((C, a, a + m))
        a += m
    assert a == NPC
    return bounds


def host_prep(positions, node_feat, w0, w1, edge_src, edge_dst, classes,
              pos_sharded):
    pos = np.ascontiguousarray(positions, dtype=np.float32)
    f = np.ascontiguousarray(node_feat, dtype=np.float32).reshape(-1)
    src = np.asarray(edge_src).astype(np.int32)
    dst = np.asarray(edge_dst).astype(np.int32)

    counts = np.bincount(dst, minlength=NT)
    try:
        # C counting sort: src values grouped by dst (stable), ~5x faster
        # than np.argsort on 3.2M keys
        from scipy import sparse
        M = sparse.coo_matrix(
            (src + 1, (dst, np.arange(len(dst), dtype=np.int32))),
            shape=(NT, len(dst))).tocsr()
        src_s = (M.data - 1).astype(np.int32)
    except ImportError:
        src_s = src[np.argsort(dst, kind="stable")]
    starts = np.zeros(NT + 1, dtype=np.int64)
    np.cumsum(counts, out=starts[1:])

    pos_pad = np.zeros((NT, 3), dtype=np.float32)
    pos_pad[:N_NODES] = pos
    f_pad = np.zeros(NT, dtype=np.float32)
    f_pad[:N_NODES] = f

    bounds = _class_layout(classes)
    chunks, cols = _chunk_plan(classes)
    wrow = np.concatenate([np.asarray(w0, np.float32).reshape(1),
                           np.asarray(w1, np.float32).reshape(3)])
    wvec = np.tile(wrow.reshape(1, 4), (P, 1)).astype(np.float32)

    in_maps = []
    perms = []
    for k in range(NC):
        lo = k * NPC
        nodes = lo + np.arange(NPC)
        d = counts[nodes]
        perm = np.argsort(-d, kind="stable")
        # feasibility: max degree per class must fit C-1 (self-slot reserved)
        for C, a, b in bounds:
            if d[perm[a]] > C - 1:
                return None, None   # caller falls back to a uniform layout
        perms.append(perm)

        ss_parts = []
        for C, a, b in bounds:
            cls_nodes = nodes[perm[a:b]]
            m = b - a
            dc = counts[cls_nodes]
            arr = np.repeat(cls_nodes, C).reshape(m, C).astype(np.int32)
            tot = int(dc.sum())
            if tot:
                l0 = np.zeros(m, dtype=np.int64)
                np.cumsum(dc[:-1], out=l0[1:])
                gidx = (np.repeat(starts[cls_nodes], dc)
                        + np.arange(tot) - np.repeat(l0, dc))
                mask = np.arange(C)[None, :] < dc[:, None]
                arr[mask] = src_s[gidx]
            nb = m // P
            ss_parts.append(
                arr.reshape(nb, P, C).transpose(1, 0, 2).reshape(P, nb * C))
        ssrc = np.concatenate(ss_parts, axis=1)
        assert ssrc.shape == (P, cols)

        stream = ssrc.T.reshape(-1)                  # i = col*128 + p
        rec_idx = (stream >> 2).astype(np.int16)
        idx_w = np.ascontiguousarray(
            rec_idx.reshape(-1, 16).T, dtype=np.int16)   # [16, len/16]
        low2 = (ssrc & 3).astype(np.uint16)
        q8 = low2.reshape(P, cols // 8, 8)
        cpk16 = np.zeros((P, cols // 8), np.uint16)
        for j in range(8):
            cpk16 |= q8[:, :, j] << np.uint16(2 * j)
        cpk16 = cpk16.view(np.int16)

        pn = nodes[perm]
        cn = np.ascontiguousarray(
            counts[pn].astype(np.int16).reshape(B, P).T)
        nf = np.ascontiguousarray(f_pad[pn].reshape(B, P).T)
        posd = pos_pad[lo:lo + NPC] if pos_sharded else pos_pad

        meg16 = np.concatenate(
            [idx_w.ravel(), cpk16.ravel(), cn.ravel()])
        megf = np.concatenate(
            [posd.ravel(), nf.ravel(), wvec.ravel()]).astype(np.float32)
        in_maps.append({"meg16": meg16, "megf": megf})
    return in_maps, perms


# ---------------------------------------------------------------------------
# cached AOT jit runner (replaces bass2jax.run_bass_via_pjrt per-call retrace)

_RUN_CACHE = {}


def _get_runner(nc):
    key = id(nc)
    if key in _RUN_CACHE:
        return _RUN_CACHE[key]

    import jax
    import jax.numpy as jnp
    from jax.sharding import Mesh, PartitionSpec, NamedSharding
    from jax.experimental.shard_map import shard_map
    from concourse import bass2jax

    bass2jax.install_neuronx_cc_hook()
    assert nc.dbg_addr is None
    partition_name = (nc.partition_id_tensor.name
                      if nc.partition_id_tensor else None)

    in_names = []
    out_names = []
    out_avals = []
    out_shapes = []
    for alloc in nc.m.functions[0].allocations:
        if not isinstance(alloc, mybir.MemoryLocationSet):
            continue
        name = alloc.memorylocations[0].name
        if alloc.kind == "ExternalInput":
            if name != partition_name:
                in_names.append(name)
        elif alloc.kind == "ExternalOutput":
            shape = tuple(alloc.tensor_shape)
            dtype = mybir.dt.np(alloc.dtype)
            out_names.append(name)
            out_avals.append(jax.core.ShapedArray(shape, dtype))
            out_shapes.append((shape, dtype))
    n_params = len(in_names)
    n_outs = len(out_avals)
    all_in_names = tuple(in_names) + tuple(out_names)
    if partition_name is not None:
        all_in_names = all_in_names + (partition_name,)
    donate = tuple(range(n_params, n_params + n_outs))

    def _body(*args):
        operands = list(args)
        if partition_name is not None:
            operands.append(bass2jax.partition_id_tensor())
        outs = bass2jax._bass_exec_p.bind(
            *operands,
            out_avals=tuple(out_avals),
            in_names=all_in_names,
            out_names=tuple(out_names),
            lowering_input_output_aliases=(),
            sim_require_finite=True,
            sim_require_nnan=True,
            nc=nc,
        )
        return tuple(outs)

    devices = jax.devices()[:NC]
    mesh = Mesh(np.asarray(devices), ("core",))
    in_specs = (PartitionSpec("core"),) * (n_params + n_outs)
    out_specs = (PartitionSpec("core"),) * n_outs
    sharded = jax.jit(
        shard_map(_body, mesh=mesh, in_specs=in_specs, out_specs=out_specs,
                  check_rep=False),
        donate_argnums=donate, keep_unused=True,
    )

    # input avals for AOT lowering: concat of the 8 per-core shapes
    fn = nc.m.functions[0]
    in_shapes = {}
    for alloc in fn.allocations:
        if (isinstance(alloc, mybir.MemoryLocationSet)
                and alloc.kind == "ExternalInput"):
            in_shapes[alloc.memorylocations[0].name] = (
                tuple(alloc.tensor_shape), mybir.dt.np(alloc.dtype))
    arg_structs = []
    for name in in_names:
        shape, dtype = in_shapes[name]
        arg_structs.append(
            jax.ShapeDtypeStruct((NC * shape[0], *shape[1:]), dtype))
    for shape, dtype in out_shapes:
        arg_structs.append(
            jax.ShapeDtypeStruct((NC * shape[0], *shape[1:]), dtype))
    compiled = sharded.lower(*arg_structs).compile()

    zero_sharding = NamedSharding(mesh, PartitionSpec("core"))

    def make_zeros():
        return [
            jax.jit(
                lambda s=shape, d=dtype: jnp.zeros((NC * s[0], *s[1:]), d),
                out_shardings=zero_sharding)()
            for shape, dtype in out_shapes
        ]

    from concurrent.futures import ThreadPoolExecutor
    pool = ThreadPoolExecutor(NC)

    entry = {"compiled": compiled, "make_zeros": make_zeros,
             "in_names": in_names, "out_names": out_names,
             # donation buffers for the next call, created outside the
             # timed path (the kernel writes every output element, so any
             # committed right-sharded buffer works; we recycle outputs)
             "donate_stash": make_zeros()}

    def run(in_maps):
        concat_in = []
        for name in in_names:
            if name in in_maps[0]:
                concat_in.append(np.concatenate(
                    [np.asarray(m[name]) for m in in_maps], axis=0))
            else:
                # framework-internal input (e.g. dbg_addr): zero-fill
                shape, dtype = in_shapes[name]
                concat_in.append(
                    np.zeros((NC * shape[0], *shape[1:]), dtype))
        donate = entry["donate_stash"] or make_zeros()
        entry["donate_stash"] = None
        out_arrs = compiled(*concat_in, *donate)
        # fetch all output shards concurrently (one axon RPC each)
        # the output is AllGathered on device: every core's shard holds the
        # full result, so fetch only device 0's shard (one RPC)
        shards = sorted(out_arrs[0].addressable_shards,
                        key=lambda s: s.index[0].start or 0)
        block0 = np.asarray(shards[0].data)      # [NC, OUT_ELEMS] fp16
        res = [{out_names[0]: block0[c]} for c in range(NC)]
        entry["donate_stash"] = list(out_arrs)
        return res

    entry["run"] = run
    _RUN_CACHE[key] = entry
    return entry


LAST_RUN_STATE = None   # (runner_entry, concat-able in_maps) for profiling


def kernel(positions, node_feat, w0, w1, edge_src, edge_dst):
    global LAST_DEVICE_WALL_S, LAST_RUN_STATE
    classes = CLASSES
    in_maps, perms = host_prep(positions, node_feat, w0, w1,
                               edge_src, edge_dst, classes, POS_SHARDED)
    if in_maps is None:
        # degree distribution outside the static plan: uniform fallback
        dst = np.asarray(edge_dst).astype(np.int32)
        maxdeg = int(np.bincount(dst, minlength=N_NODES).max())
        C = ((maxdeg + 1 + 7) // 8) * 8    # +1: self-slot reserved
        classes = ((max(C, 8), B),)
        in_maps, perms = host_prep(positions, node_feat, w0, w1,
                                   edge_src, edge_dst, classes, POS_SHARDED)
        assert in_maps is not None

    nc = build_program(classes, POS_SHARDED)
    runner = _get_runner(nc)
    LAST_RUN_STATE = (runner, in_maps)
    t0 = time.perf_counter()
    res = runner["run"](in_maps)
    LAST_DEVICE_WALL_S = time.perf_counter() - t0

    full = np.zeros((NT, 4), dtype=np.float32)
    for k in range(NC):
        o = res[k]["out"].reshape(128, B, 4)   # fp16, x4096
        o_flat = o.transpose(1, 0, 2).reshape(NPC, 4)
        full[k * NPC + perms[k]] = o_flat.astype(np.float32) * np.float32(2.0 ** -12)
    return full[:N_NODES]


# revision 24
# speedup vs baseline: 1.0614x; 1.0614x over previous
"""TRN2 Bass kernel for gnn_message_passing (nn_Model_34823594836411), v2.

Math (matches reference.py):
  per edge e: rel = pos[dst] - pos[src]; sh1 = rel / max(|rel|, 1e-12)
  out[n, 0]   = w0 * f[n] * min(c_n, 1)
  out[n, 1:4] = w1 * f[n] * segsum(sh1)_n / max(c_n, 1)
where f = node_feat[:, 0] and c_n = in-degree of node n.

Design (the end-to-end wall is dominated by the ~50-85 MB/s axon host->
device link, so everything minimizes shipped bytes; on-device exec is
~1.5ms):
  * dst-shard nodes across 8 cores (12544/core). Each node owns a padded
    row of C slots; the only random access is the src-position gather via
    the ANT dma_gather SWDGE ucode over 256B-stride records of 4 nodes.
  * The 6.4MB/core gather table is NOT shipped: positions are shipped
    densely, 1/8-sharded per core (150KB), AllGathered on device, and the
    record table is built on device (vector 12->64 expand + one DMA).
  * Nodes are degree-sorted per core into capacity classes
    (C in {72,48,40,36,32,28} instead of uniform 64), cutting padded
    slots/node from 64 to ~35.7. Host un-permutes the output.
  * Slot C-1 of every row is a self-edge, so the on-chip select also
    yields the node's own position (no pdst input); self slots contribute
    exactly zero (rel=0 -> sh=0).
  * 2-bit record-select codes ship packed 4-per-byte; in-degrees ship as
    uint16; both are unpacked/converted on device.
  * The jit executable is AOT-compiled once and cached; output donation
    buffers are created on device (never shipped) and recycled across
    calls; output shards are fetched with parallel RPCs.
"""
import time
from contextlib import ExitStack

import numpy as np

import concourse.bacc as bacc
import concourse.bass as bass
import concourse.mybir as mybir
from concourse import library_config
from concourse._compat import exact_div

N_NODES = 100000
N_EDGES = 3200000
NC = 8
P = 128
NPC = 12544            # nodes per core (98 blocks of 128); 8*12544 = 100352
B = NPC // P           # 98 blocks
NT = NC * NPC          # padded node table size
NREC = NT // 4         # 4-node records in the position table
EPS2 = 1e-24
CALL_IDX = 1024        # gather idxs per dma_gather call (ring-capacity safe)
MAX_CH_COLS = 896      # SBUF budget for per-chunk tiles
# (capacity C, num 128-node blocks) from highest degree to lowest; sum of
# blocks must be B. Slot C-1 of every node is reserved as a self-edge (so
# the on-chip select also yields the node's own position -> no pdst input);
# usable capacity is C-1. Sized for Poisson(32) degrees with >=5 sigma
# margin at every boundary.
CLASSES = ((72, 1), (48, 10), (40, 16), (36, 26), (32, 27), (28, 18))
POS_SHARDED = True     # ship positions 1/8-sharded + AllGather on device


def set_mini(n_nodes, nc_, npc, classes, call_idx=128, max_ch_cols=64):
    """Shrink the problem for CoreSim debugging."""
    global N_NODES, NC, NPC, B, NT, NREC, CLASSES, CALL_IDX, MAX_CH_COLS
    N_NODES, NC, NPC = n_nodes, nc_, npc
    B = NPC // P
    NT = NC * NPC
    NREC = NT // 4
    CLASSES = classes
    CALL_IDX = call_idx
    MAX_CH_COLS = max_ch_cols


F32 = mybir.dt.float32
I16 = mybir.dt.int16
U8 = mybir.dt.uint8


def _ap(t, off, dims):
    return bass.AP(t, off, dims)


def _chunk_plan(classes):
    """Static chunk descriptors: (C, nb, bg0, col0, cols)."""
    chunks = []
    bg = 0
    col = 0
    for C, nblocks in classes:
        left = nblocks
        while left > 0:
            nb = min(left, MAX_CH_COLS // C)
            if C % 8 and nb % 2 and nb > 1:
                nb -= 1          # keep nb*C a multiple of 8 (CALL_IDX slicing)
            assert nb >= 1 and (nb * C) % 8 == 0, (C, nb)
            chunks.append((C, nb, bg, col, nb * C))
            bg += nb
            col += nb * C
            left -= nb
    assert bg == sum(nb for _, nb in classes)
    return chunks, col


def dma_gather_raw(gpsimd, out_ap, in_ap, idxs_ap, num_idxs, elem_size,
                   elem_step, queue_num=0):
    """Non-transpose DRAM-source InstDMAGatherAnt without the 256B-elem
    assert: out[i % 128, i // 128, :] = table[idx[i], :elem_size]."""
    stride_bytes_256 = exact_div(elem_step * 4, 256)
    return gpsimd.add_instruction(
        mybir.InstDMAGatherAnt(
            name=gpsimd.bass.get_next_instruction_name(),
            ins=[
                *gpsimd.lower_ap_dma(in_ap, for_custom_bir_dma=True),
                gpsimd.lower_ap(idxs_ap),
                gpsimd.lower_val_access(gpsimd.to_reg(num_idxs)),
            ],
            outs=[gpsimd.lower_ap(out_ap)],
            transpose=False,
            num_idxs=num_idxs,
            elem_size=elem_size,
            stride_bytes_256=stride_bytes_256,
            gen_mode=0,
            single_packet=True,
            queue_num=queue_num,
            sbuf_tokens_per_rank=0,
            sbuf_free_dim_per_rank=0,
            sbuf_free_dim_pad_per_rank=0,
            sbuf_byte_offset=0,
        )
    )


_PROG_CACHE = {}
LAST_DEVICE_WALL_S = None


def build_program(classes, pos_sharded):
    key = (classes, pos_sharded, NPC, CALL_IDX, MAX_CH_COLS)
    if key in _PROG_CACHE:
        return _PROG_CACHE[key]

    AL = mybir.AluOpType
    chunks, cols = _chunk_plan(classes)
    max_cols = max(c[4] for c in chunks)
    pf_cols = exact_div(NT * 3, P)   # dense position table as a [128, x] tile
    qrec = exact_div(NREC, P)        # records per partition in the build image

    nc = bacc.Bacc("TRN2", num_swdge_queues=4, num_devices=NC)
    _eps_t = nc.alloc_sbuf_tensor("const-float32-eps2", [128, 1], F32)
    nc.gpsimd.memset(_eps_t.ap(), EPS2)
    nc.const_aps.aps[(F32, EPS2)] = _eps_t.ap()
    nc.all_engine_barrier()

    # all inputs ship as TWO per-core arrays (one int16, one f32) to
    # minimize per-array host->device placement overhead over axon.
    # int16 layout: [16*W idx stream | 128*(cols//8) packed codes | 128*B
    # degrees]; f32 layout: [positions | 128*B node_feat | 128*4 weights]
    W = cols * P // 16
    OFF_B = 16 * W
    OFF_C = OFF_B + 128 * (cols // 8)
    M16 = OFF_C + 128 * B
    plen = (NPC if pos_sharded else NT) * 3
    OFF_N = plen
    OFF_W = OFF_N + 128 * B
    MF = OFF_W + 128 * 4
    meg16 = nc.dram_tensor("meg16", [M16], I16, kind="ExternalInput")
    megf = nc.dram_tensor("megf", [MF], F32, kind="ExternalInput")
    if pos_sharded:
        pbounce = nc.dram_tensor("pbounce", [NPC, 3], F32)
        pfull = nc.dram_tensor("pfull", [NT, 3], F32)
    else:
        pfull = None                    # dense positions read from megf
    ptab = nc.dram_tensor("ptab", [NREC, 64], F32)
    # output ships fp16 scaled by 2^12 (exact) to halve the fetch bytes;
    # host unscales. fp16 rounding is a bounded ~5e-4 RELATIVE error (no
    # cancellation amplification), far inside the 2e-2 gate; the scale
    # keeps all representable values out of subnormal-flush range.
    out = nc.dram_tensor("out", [128, B, 4], mybir.dt.float16,
                         kind="ExternalOutput")

    tab_ap = _ap(ptab, 0, [[64, NREC], [1, 12]])

    # deterministic semaphore schedule, computed identically for every engine
    n_chunks = len(chunks)
    calls_of = [exact_div(c[4] * P, CALL_IDX) for c in chunks]
    g_setup = 16 * (6 if pos_sharded else 5)   # setup DMAs (see gpsimd below)
    g_after = []
    q_after = []
    g = g_setup
    qc = [0, 0, 0, 0]
    for ch in range(n_chunks):
        g += 16 * 9                      # 8 idx-group DMAs + cpk DMA
        g_after.append(g)
        for k in range(calls_of[ch]):
            qc[k % 4] += 16
        q_after.append(tuple(qc))

    with ExitStack() as _st:
        idx_sb = _st.enter_context(
            nc.sbuf_tensor("idx_sb", [128, max_cols * P // 16], I16))
        sbE = _st.enter_context(nc.sbuf_tensor("sbE", [128, qrec, 64], F32))
        cpk_sb = _st.enter_context(nc.sbuf_tensor("cpk_sb", [128, max_cols // 8], I16))
        cdt_sb = _st.enter_context(nc.sbuf_tensor("cdt_sb", [128, max_cols // 8], I16))
        cd_sb = _st.enter_context(nc.sbuf_tensor("cd_sb", [128, max_cols], I16))
        rec_sb = _st.enter_context(nc.sbuf_tensor("rec_sb", [128, max_cols, 12], F32))
        mk_sb = _st.enter_context(nc.sbuf_tensor("mk_sb", [128, 4, max_cols], F32))
        pa_sb = _st.enter_context(nc.sbuf_tensor("pa_sb", [128, max_cols, 3], F32))
        pb_sb = _st.enter_context(nc.sbuf_tensor("pb_sb", [128, max_cols, 3], F32))
        ss_sb = _st.enter_context(nc.sbuf_tensor("ss_sb", [128, max_cols], F32))
        inv_sb = _st.enter_context(nc.sbuf_tensor("inv_sb", [128, max_cols], F32))
        pdst_sb = _st.enter_context(nc.sbuf_tensor("pdst_sb", [128, B, 3], F32))
        sums_sb = _st.enter_context(nc.sbuf_tensor("sums_sb", [128, B, 3], F32))
        cnt_sb = _st.enter_context(nc.sbuf_tensor("cnt_sb", [128, B], I16))
        cntf_sb = _st.enter_context(nc.sbuf_tensor("cntf_sb", [128, B], F32))
        nf_sb = _st.enter_context(nc.sbuf_tensor("nf_sb", [128, B], F32))
        w_sb = _st.enter_context(nc.sbuf_tensor("w_sb", [128, 4], F32))
        o_sb = _st.enter_context(nc.sbuf_tensor("o_sb", [128, B, 4], F32))
        o16_sb = _st.enter_context(
            nc.sbuf_tensor("o16_sb", [128, B, 4], mybir.dt.float16))
        t0_sb = _st.enter_context(nc.sbuf_tensor("t0_sb", [128, B], F32))
        t1_sb = _st.enter_context(nc.sbuf_tensor("t1_sb", [128, B], F32))
        g_sem = _st.enter_context(nc.semaphore("g_sem"))
        q0_sem = _st.enter_context(nc.semaphore("q0_sem"))
        q1_sem = _st.enter_context(nc.semaphore("q1_sem"))
        q2_sem = _st.enter_context(nc.semaphore("q2_sem"))
        q3_sem = _st.enter_context(nc.semaphore("q3_sem"))
        v_sem = _st.enter_context(nc.semaphore("v_sem"))
        a_sem = _st.enter_context(nc.semaphore("a_sem"))
        c_sem = _st.enter_context(nc.semaphore("c_sem"))
        b_sem = _st.enter_context(nc.semaphore("b_sem"))
        block = _st.enter_context(nc.Block())

        # dense pfull viewed as a [128, pf_cols] SBUF tile in linear order:
        # partition p holds f32 elements [p*pf_cols, (p+1)*pf_cols)
        pf_flat_sb = _ap(pa_sb, 0, [[max_cols * 3, 128], [1, pf_cols]])
        pf_flat_dram = _ap(pfull if pos_sharded else megf, 0,
                           [[pf_cols, 128], [1, pf_cols]])
        # table build: vector expands 12->64 f32/record into sbE (partition p
        # holds records [p*qrec, (p+1)*qrec)), then one fully-contiguous DMA
        # writes ptab (row r payload lands at 256B*r; cols 12..63 are
        # garbage the gather never reads)
        exp_src = _ap(pa_sb, 0, [[max_cols * 3, 128], [12, qrec], [1, 12]])
        exp_dst = _ap(sbE, 0, [[qrec * 64, 128], [64, qrec], [1, 12]])
        tab_dst = _ap(ptab, 0, [[qrec * 64, 128], [1, qrec * 64]])
        tab_src = _ap(sbE, 0, [[qrec * 64, 128], [1, qrec * 64]])

        @block.gpsimd
        def _(gpsimd):
            gpsimd.load_library(library_config.mlp)
            ng = 0
            if pos_sharded:
                gpsimd.dma_start(
                    pbounce[:], _ap(megf, 0, [[3, NPC], [1, 3]])
                ).then_inc(g_sem, 16)
                ng += 16
                gpsimd.wait_ge(g_sem, ng)
                gpsimd.collective_compute(
                    "AllGather",
                    AL.bypass,
                    replica_groups=[list(range(NC))],
                    ins=[pbounce.ap().opt()],
                    outs=[pfull.ap().opt()],
                ).then_inc(c_sem, 1)
                gpsimd.wait_ge(c_sem, 1)
            gpsimd.dma_start(pf_flat_sb, pf_flat_dram).then_inc(g_sem, 16)
            ng += 16
            gpsimd.wait_ge(b_sem, 1)        # vector expanded sbE
            gpsimd.dma_start(tab_dst, tab_src).then_inc(g_sem, 16)
            ng += 16
            gpsimd.dma_start(
                cnt_sb[:], _ap(meg16, OFF_C, [[B, 128], [1, B]])
            ).then_inc(g_sem, 16)
            gpsimd.dma_start(
                nf_sb[:], _ap(megf, OFF_N, [[B, 128], [1, B]])
            ).then_inc(g_sem, 16)
            gpsimd.dma_start(
                w_sb[:], _ap(megf, OFF_W, [[4, 128], [1, 4]])
            ).then_inc(g_sem, 16)
            ng += 16 * 3
            assert ng == g_setup
            # all setup DMAs (incl. the table build write) must land before
            # the first gather reads ptab / vector overwrites pa_sb
            gpsimd.wait_ge(g_sem, g_setup)
            q_sems = (q0_sem, q1_sem, q2_sem, q3_sem)
            for ch, (C, nb, bg0, col0, ccols) in enumerate(chunks):
                if ch >= 1:
                    # chunk buffers are single-buffered: wait for compute
                    gpsimd.wait_ge(v_sem, ch)
                iw = ccols * P // 16
                ioff = col0 * P // 16
                for grp in range(8):
                    # replicate the wrapped idx stream into each
                    # 16-partition group on device
                    gpsimd.dma_start(
                        idx_sb[16 * grp:16 * (grp + 1), :iw],
                        _ap(meg16, ioff, [[W, 16], [1, iw]]),
                    ).then_inc(g_sem, 16)
                gpsimd.dma_start(
                    cpk_sb[:, :ccols // 8],
                    _ap(meg16, OFF_B + col0 // 8,
                        [[cols // 8, 128], [1, ccols // 8]]),
                ).then_inc(g_sem, 16)
                gpsimd.wait_ge(g_sem, g_after[ch])
                ncalls = calls_of[ch]
                ccall = CALL_IDX // P    # record columns written per call
                for k in range(ncalls):
                    dma_gather_raw(
                        gpsimd,
                        rec_sb[:, k * ccall:(k + 1) * ccall, :],
                        tab_ap,
                        idx_sb[:, k * (CALL_IDX // 16):(k + 1) * (CALL_IDX // 16)],
                        num_idxs=CALL_IDX, elem_size=12, elem_step=64,
                        queue_num=k % 4,
                    ).then_inc(q_sems[k % 4], 16)
            gpsimd.wait_ge(v_sem, n_chunks + 1)
            gpsimd.dma_start(out[:], o16_sb[:]).then_inc(g_sem, 16)
            gpsimd.wait_ge(g_sem, g_after[-1] + 16)
            for qi, q in enumerate(q_sems):
                gpsimd.wait_ge(q, q_after[-1][qi])

        @block.vector
        def _(vector):
            # expand the dense positions into the 256B-stride record image
            vector.memset(sbE[:], 0.0)
            vector.drain()
            vector.wait_ge(g_sem, 32 if pos_sharded else 16)
            vector.tensor_copy(out=exp_dst, in_=exp_src)
            vector.drain().then_inc(b_sem, 1)
            for ch, (C, nb, bg0, col0, ccols) in enumerate(chunks):
                vector.wait_ge(g_sem, g_after[ch])
                for qi, q in enumerate((q0_sem, q1_sem, q2_sem, q3_sem)):
                    if q_after[ch][qi]:
                        vector.wait_ge(q, q_after[ch][qi])
                qcols = ccols // 8
                # unpack the 2-bit codes: cd[:, 8q+r] = (cpk[:, q] >> 2r) & 3
                for r in range(8):
                    srcap = cpk_sb[:, :qcols]
                    if r > 0:
                        vector.tensor_scalar(
                            out=cdt_sb[:, :qcols], in0=cpk_sb[:, :qcols],
                            scalar1=2 * r, scalar2=None,
                            op0=AL.logical_shift_right)
                        vector.drain()
                        srcap = cdt_sb[:, :qcols]
                    vector.tensor_scalar(
                        out=_ap(cd_sb, r, [[max_cols, 128], [8, qcols]]),
                        in0=srcap, scalar1=3, scalar2=None,
                        op0=AL.bitwise_and)
                    vector.drain()
                # derive the four 0/1 masks from the code plane
                for kk in range(4):
                    vector.tensor_scalar(
                        out=_ap(mk_sb, kk * max_cols,
                                [[4 * max_cols, 128], [1, ccols]]),
                        in0=cd_sb[:, :ccols], scalar1=kk, scalar2=None,
                        op0=AL.is_equal)
                vector.drain()
                # exact select: psrc = sum_k rec_k * mask_k (three terms are
                # exact zeros, so the sum is bit-exact)
                def mk(kk):
                    return _ap(mk_sb, kk * max_cols,
                               [[4 * max_cols, 128], [1, ccols], [0, 3]])
                vector.tensor_tensor(out=pa_sb[:, :ccols, :],
                                     in0=rec_sb[:, :ccols, 0:3],
                                     in1=mk(0), op=AL.mult)
                for kk in range(1, 4):
                    vector.tensor_tensor(out=pb_sb[:, :ccols, :],
                                         in0=rec_sb[:, :ccols, 3 * kk:3 * kk + 3],
                                         in1=mk(kk), op=AL.mult)
                    vector.drain()
                    vector.tensor_tensor(out=pa_sb[:, :ccols, :],
                                         in0=pa_sb[:, :ccols, :],
                                         in1=pb_sb[:, :ccols, :],
                                         op=AL.add)
                    vector.drain()
                # slot C-1 is a self-edge: psrc there is this node's own
                # position; stash it as pdst for the chunk's blocks
                vector.tensor_copy(
                    out=_ap(pdst_sb, bg0 * 3, [[B * 3, 128], [3, nb], [1, 3]]),
                    in_=_ap(pa_sb, (C - 1) * 3,
                            [[max_cols * 3, 128], [C * 3, nb], [1, 3]]))
                vector.drain()
                # rel = pdst - psrc (in place, 4D APs)
                pd = _ap(pdst_sb, bg0 * 3,
                         [[B * 3, 128], [3, nb], [0, C], [1, 3]])
                pa4 = _ap(pa_sb, 0,
                          [[max_cols * 3, 128], [C * 3, nb], [3, C], [1, 3]])
                vector.tensor_tensor(out=pa4, in0=pd, in1=pa4, op=AL.subtract)
                vector.drain()
                # ss = sum of squares over components
                vector.tensor_tensor(out=pb_sb[:, :ccols, :],
                                     in0=pa_sb[:, :ccols, :],
                                     in1=pa_sb[:, :ccols, :], op=AL.mult)
                vector.drain()
                sq_x = _ap(pb_sb, 0, [[max_cols * 3, 128], [3, ccols]])
                sq_y = _ap(pb_sb, 1, [[max_cols * 3, 128], [3, ccols]])
                sq_z = _ap(pb_sb, 2, [[max_cols * 3, 128], [3, ccols]])
                vector.tensor_tensor(out=ss_sb[:, :ccols], in0=sq_x, in1=sq_y,
                                     op=AL.add)
                vector.drain()
                vector.tensor_tensor(out=ss_sb[:, :ccols], in0=ss_sb[:, :ccols],
                                     in1=sq_z, op=AL.add)
                vector.drain().then_inc(a_sem, 1)
                # sh = rel * rsqrt(ss + eps^2) once ACT publishes inv
                vector.wait_ge(a_sem, 2 * ch + 2)
                vector.reciprocal(out=inv_sb[:, :ccols], in_=inv_sb[:, :ccols])
                vector.drain()
                invb = _ap(inv_sb, 0, [[max_cols, 128], [1, ccols], [0, 3]])
                vector.tensor_tensor(out=pa_sb[:, :ccols, :],
                                     in0=pa_sb[:, :ccols, :], in1=invb,
                                     op=AL.mult)
                vector.drain()
                # reduce over C: halve while even, else fold last col into 0
                width = C
                while width > 1:
                    if width % 2 == 0:
                        half = width // 2
                        a_lo = _ap(pa_sb, 0,
                                   [[max_cols * 3, 128], [C * 3, nb],
                                    [3, half], [1, 3]])
                        a_hi = _ap(pa_sb, half * 3,
                                   [[max_cols * 3, 128], [C * 3, nb],
                                    [3, half], [1, 3]])
                        vector.tensor_tensor(out=a_lo, in0=a_lo, in1=a_hi,
                                             op=AL.add)
                        vector.drain()
                        width = half
                    else:
                        a_l0 = _ap(pa_sb, 0,
                                   [[max_cols * 3, 128], [C * 3, nb], [1, 3]])
                        a_lst = _ap(pa_sb, (width - 1) * 3,
                                    [[max_cols * 3, 128], [C * 3, nb], [1, 3]])
                        vector.tensor_tensor(out=a_l0, in0=a_l0, in1=a_lst,
                                             op=AL.add)
                        vector.drain()
                        width -= 1
                dst_sums = _ap(sums_sb, bg0 * 3,
                               [[B * 3, 128], [3, nb], [1, 3]])
                src_sums = _ap(pa_sb, 0,
                               [[max_cols * 3, 128], [C * 3, nb], [1, 3]])
                vector.tensor_copy(out=dst_sums, in_=src_sums)
                vector.drain().then_inc(v_sem, 1)
            # final combine
            vector.tensor_copy(out=cntf_sb[:], in_=cnt_sb[:])
            vector.drain()
            vector.tensor_scalar_min(out=t0_sb[:], in0=cntf_sb[:], scalar1=1.0)
            vector.tensor_scalar_max(out=t1_sb[:], in0=cntf_sb[:], scalar1=1.0)
            vector.drain()
            vector.reciprocal(out=t1_sb[:], in_=t1_sb[:])
            vector.drain()
            vector.tensor_tensor(out=t1_sb[:], in0=t1_sb[:], in1=nf_sb[:],
                                 op=AL.mult)
            vector.drain()
            o0 = _ap(o_sb, 0, [[B * 4, 128], [4, B]])
            w0b = _ap(w_sb, 0, [[4, 128], [0, B]])
            vector.tensor_tensor(out=o0, in0=t0_sb[:], in1=nf_sb[:], op=AL.mult)
            vector.drain()
            vector.tensor_tensor(out=o0, in0=o0, in1=w0b, op=AL.mult)
            vector.drain()
            for c in range(3):
                oc = _ap(o_sb, 1 + c, [[B * 4, 128], [4, B]])
                sc = _ap(sums_sb, c, [[B * 3, 128], [3, B]])
                wcb = _ap(w_sb, 1 + c, [[4, 128], [0, B]])
                vector.tensor_tensor(out=oc, in0=sc, in1=t1_sb[:], op=AL.mult)
                vector.drain()
                vector.tensor_tensor(out=oc, in0=oc, in1=wcb, op=AL.mult)
                vector.drain()
            # scaled fp16 cast for the half-size output fetch
            vector.tensor_scalar(out=o16_sb[:], in0=o_sb[:],
                                 scalar1=4096.0, scalar2=None, op0=AL.mult)
            vector.drain().then_inc(v_sem, 1)

        @block.scalar
        def _(scalar):
            for ch, (C, nb, bg0, col0, ccols) in enumerate(chunks):
                scalar.wait_ge(a_sem, 2 * ch + 1)
                scalar.activation(
                    out=inv_sb[:, :ccols], in_=ss_sb[:, :ccols],
                    func=mybir.ActivationFunctionType.Sqrt,
                    bias=EPS2, scale=1.0,
                ).then_inc(a_sem, 1)

    nc.compile()
    _PROG_CACHE[key] = nc
    return nc


def _class_layout(classes):
    bounds = []
    a = 0
    for C, nblocks in classes:
        m = nblocks * P
        bounds.append((C, a, a + m))
        a += m
    assert a == NPC
    return bounds


def host_prep(positions, node_feat, w0, w1, edge_src, edge_dst, classes,
              pos_sharded):
    pos = np.ascontiguousarray(positions, dtype=np.float32)
    f = np.ascontiguousarray(node_feat, dtype=np.float32).reshape(-1)
    src = np.asarray(edge_src).astype(np.int32)
    dst = np.asarray(edge_dst).astype(np.int32)

    counts = np.bincount(dst, minlength=NT)
    try:
        # C counting sort: src values grouped by dst (stable), ~5x faster
        # than np.argsort on 3.2M keys
        from scipy import sparse
        M = sparse.coo_matrix(
            (src + 1, (dst, np.arange(len(dst), dtype=np.int32))),
            shape=(NT, len(dst))).tocsr()
        src_s = (M.data - 1).astype(np.int32)
    except ImportError:
        src_s = src[np.argsort(dst, kind="stable")]
    starts = np.zeros(NT + 1, dtype=np.int64)
    np.cumsum(counts, out=starts[1:])

    pos_pad = np.zeros((NT, 3), dtype=np.float32)
    pos_pad[:N_NODES] = pos
    f_pad = np.zeros(NT, dtype=np.float32)
    f_pad[:N_NODES] = f

    bounds = _class_layout(classes)
    chunks, cols = _chunk_plan(classes)
    wrow = np.concatenate([np.asarray(w0, np.float32).reshape(1),
                           np.asarray(w1, np.float32).reshape(3)])
    wvec = np.tile(wrow.reshape(1, 4), (P, 1)).astype(np.float32)

    in_maps = []
    perms = []
    for k in range(NC):
        lo = k * NPC
        nodes = lo + np.arange(NPC)
        d = counts[nodes]
        perm = np.argsort(-d, kind="stable")
        # feasibility: max degree per class must fit C-1 (self-slot reserved)
        for C, a, b in bounds:
            if d[perm[a]] > C - 1:
                return None, None   # caller falls back to a uniform layout
        perms.append(perm)

        ss_parts = []
        for C, a, b in bounds:
            cls_nodes = nodes[perm[a:b]]
            m = b - a
            dc = counts[cls_nodes]
            arr = np.repeat(cls_nodes, C).reshape(m, C).astype(np.int32)
            tot = int(dc.sum())
            if tot:
                l0 = np.zeros(m, dtype=np.int64)
                np.cumsum(dc[:-1], out=l0[1:])
                gidx = (np.repeat(starts[cls_nodes], dc)
                        + np.arange(tot) - np.repeat(l0, dc))
                mask = np.arange(C)[None, :] < dc[:, None]
                arr[mask] = src_s[gidx]
            nb = m // P
            ss_parts.append(
                arr.reshape(nb, P, C).transpose(1, 0, 2).reshape(P, nb * C))
        ssrc = np.concatenate(ss_parts, axis=1)
        assert ssrc.shape == (P, cols)

        stream = ssrc.T.reshape(-1)                  # i = col*128 + p
        rec_idx = (stream >> 2).astype(np.int16)
        idx_w = np.ascontiguousarray(
            rec_idx.reshape(-1, 16).T, dtype=np.int16)   # [16, len/16]
        low2 = (ssrc & 3).astype(np.uint16)
        q8 = low2.reshape(P, cols // 8, 8)
        cpk16 = np.zeros((P, cols // 8), np.uint16)
        for j in range(8):
            cpk16 |= q8[:, :, j] << np.uint16(2 * j)
        cpk16 = cpk16.view(np.int16)

        pn = nodes[perm]
        cn = np.ascontiguousarray(
            counts[pn].astype(np.int16).reshape(B, P).T)
        nf = np.ascontiguousarray(f_pad[pn].reshape(B, P).T)
        posd = pos_pad[lo:lo + NPC] if pos_sharded else pos_pad

        meg16 = np.concatenate(
            [idx_w.ravel(), cpk16.ravel(), cn.ravel()])
        megf = np.concatenate(
            [posd.ravel(), nf.ravel(), wvec.ravel()]).astype(np.float32)
        in_maps.append({"meg16": meg16, "megf": megf})
    return in_maps, perms


# ---------------------------------------------------------------------------
# cached AOT jit runner (replaces bass2jax.run_bass_via_pjrt per-call retrace)

_RUN_CACHE = {}


def _get_runner(nc):
    key = id(nc)
    if key in _RUN_CACHE:
        return _RUN_CACHE[key]

    import jax
    import jax.numpy as jnp
    from jax.sharding import Mesh, PartitionSpec, NamedSharding
    from jax.experimental.shard_map import shard_map
    from concourse import bass2jax

    bass2jax.install_neuronx_cc_hook()
    assert nc.dbg_addr is None
    partition_name = (nc.partition_id_tensor.name
                      if nc.partition_id_tensor else None)

    in_names = []
    out_names = []
    out_avals = []
    out_shapes = []
    for alloc in nc.m.functions[0].allocations:
        if not isinstance(alloc, mybir.MemoryLocationSet):
            continue
        name = alloc.memorylocations[0].name
        if alloc.kind == "ExternalInput":
            if name != partition_name:
                in_names.append(name)
        elif alloc.kind == "ExternalOutput":
            shape = tuple(alloc.tensor_shape)
            dtype = mybir.dt.np(alloc.dtype)
            out_names.append(name)
            out_avals.append(jax.core.ShapedArray(shape, dtype))
            out_shapes.append((shape, dtype))
    n_params = len(in_names)
    n_outs = len(out_avals)
    all_in_names = tuple(in_names) + tuple(out_names)
    if partition_name is not None:
        all_in_names = all_in_names + (partition_name,)
    donate = tuple(range(n_params, n_params + n_outs))

    def _body(*args):
        operands = list(args)
        if partition_name is not None:
            operands.append(bass2jax.partition_id_tensor())
        outs = bass2jax._bass_exec_p.bind(
            *operands,
            out_avals=tuple(out_avals),
            in_names=all_in_names,
            out_names=tuple(out_names),
            lowering_input_output_aliases=(),
            sim_require_finite=True,
            sim_require_nnan=True,
            nc=nc,
        )
        return tuple(outs)

    devices = jax.devices()[:NC]
    mesh = Mesh(np.asarray(devices), ("core",))
    in_specs = (PartitionSpec("core"),) * (n_params + n_outs)
    out_specs = (PartitionSpec("core"),) * n_outs
    sharded = jax.jit(
        shard_map(_body, mesh=mesh, in_specs=in_specs, out_specs=out_specs,
                  check_rep=False),
        donate_argnums=donate, keep_unused=True,
    )

    # input avals for AOT lowering: concat of the 8 per-core shapes
    fn = nc.m.functions[0]
    in_shapes = {}
    for alloc in fn.allocations:
        if (isinstance(alloc, mybir.MemoryLocationSet)
                and alloc.kind == "ExternalInput"):
            in_shapes[alloc.memorylocations[0].name] = (
                tuple(alloc.tensor_shape), mybir.dt.np(alloc.dtype))
    arg_structs = []
    for name in in_names:
        shape, dtype = in_shapes[name]
        arg_structs.append(
            jax.ShapeDtypeStruct((NC * shape[0], *shape[1:]), dtype))
    for shape, dtype in out_shapes:
        arg_structs.append(
            jax.ShapeDtypeStruct((NC * shape[0], *shape[1:]), dtype))
    compiled = sharded.lower(*arg_structs).compile()

    zero_sharding = NamedSharding(mesh, PartitionSpec("core"))

    def make_zeros():
        return [
            jax.jit(
                lambda s=shape, d=dtype: jnp.zeros((NC * s[0], *s[1:]), d),
                out_shardings=zero_sharding)()
            for shape, dtype in out_shapes
        ]

    from concurrent.futures import ThreadPoolExecutor
    pool = ThreadPoolExecutor(NC)

    entry = {"compiled": compiled, "make_zeros": make_zeros,
             "in_names": in_names, "out_names": out_names,
             # donation buffers for the next call, created outside the
             # timed path (the kernel writes every output element, so any
             # committed right-sharded buffer works; we recycle outputs)
             "donate_stash": make_zeros()}

    def run(in_maps):
        concat_in = []
        for name in in_names:
            if name in in_maps[0]:
                concat_in.append(np.concatenate(
                    [np.asarray(m[name]) for m in in_maps], axis=0))
            else:
                # framework-internal input (e.g. dbg_addr): zero-fill
                shape, dtype = in_shapes[name]
                concat_in.append(
                    np.zeros((NC * shape[0], *shape[1:]), dtype))
        donate = entry["donate_stash"] or make_zeros()
        entry["donate_stash"] = None
        out_arrs = compiled(*concat_in, *donate)
        # fetch all output shards concurrently (one axon RPC each)
        per_out = []
        for i, name in enumerate(out_names):
            shards = sorted(out_arrs[i].addressable_shards,
                            key=lambda s: s.index[0].start or 0)
            per_out.append(list(pool.map(
                lambda s: np.asarray(s.data), shards)))
        res = [
            {name: per_out[i][c] for i, name in enumerate(out_names)}
            for c in range(NC)
        ]
        entry["donate_stash"] = list(out_arrs)
        return res

    entry["run"] = run
    _RUN_CACHE[key] = entry
    return entry


LAST_RUN_STATE = None   # (runner_entry, concat-able in_maps) for profiling


def kernel(positions, node_feat, w0, w1, edge_src, edge_dst):
    global LAST_DEVICE_WALL_S, LAST_RUN_STATE
    classes = CLASSES
    in_maps, perms = host_prep(positions, node_feat, w0, w1,
                               edge_src, edge_dst, classes, POS_SHARDED)
    if in_maps is None:
        # degree distribution outside the static plan: uniform fallback
        dst = np.asarray(edge_dst).astype(np.int32)
        maxdeg = int(np.bincount(dst, minlength=N_NODES).max())
        C = ((maxdeg + 1 + 7) // 8) * 8    # +1: self-slot reserved
        classes = ((max(C, 8), B),)
        in_maps, perms = host_prep(positions, node_feat, w0, w1,
                                   edge_src, edge_dst, classes, POS_SHARDED)
        assert in_maps is not None

    nc = build_program(classes, POS_SHARDED)
    runner = _get_runner(nc)
    LAST_RUN_STATE = (runner, in_maps)
    t0 = time.perf_counter()
    res = runner["run"](in_maps)
    LAST_DEVICE_WALL_S = time.perf_counter() - t0

    full = np.zeros((NT, 4), dtype=np.float32)
    for k in range(NC):
        o = res[k]["out"]                      # [128, B, 4] fp16, x4096
        o_flat = o.transpose(1, 0, 2).reshape(NPC, 4)
        full[k * NPC + perms[k]] = o_flat.astype(np.float32) * np.float32(2.0 ** -12)
    return full[:N_NODES]
